# revision 15
# baseline (speedup 1.0000x reference)
"""Trainium2 Bass kernel for nn_AttnMixer (2D-local sparse attention).

Strategy (v2): data-parallel over batch N=32 across 8 cores (4/core).
W-MAJOR spatial layout: position p = w*16 + h, so the |dw|<=5 band becomes
an 18-column window -> 288-key (384 padded) windows instead of 512, and the
|dh|<=3 band lives entirely inside each 16-high column (pure mask, no
windowing).

Per (q-tile of 128 = 8 w-cols, head-pair): scores are computed TRANSPOSED
(k-major, [kwin, 128q] PSUM chunks) so AV consumes the probabilities
directly -- no PE transposes and no PSUM->SBUF prob copies. exp on ACT
(bias -2), 0/1-mask multiply on Pool, AV matmuls accumulate o^T per head
into a shared PSUM tile, softmax denominators z ride as [1,128] ones-vector
matmuls into spare rows of the same tile, reciprocal on DVE, 1/z broadcast
across partitions via one K=2 matmul, and the normalize is fused into the
single PSUM->SBUF merge per head-pair on DVE. All matmul dtypes fp16
(fp8 tested: exceeds the 2e-2 error budget).
"""

import numpy as np

N_CORES = 8
N, S, C = 32, 1024, 512
NB = N // N_CORES
GH, GW = 16, 64
HK, WK = 7, 11
NH, HD = 8, 64
NT = S // 128  # 8 q-tiles (8 w-cols each)


def _win(t):
    # key window for q-tile t (q w-cols [8t, 8t+8); valid keys w in
    # [8t-5, 8t+13)). Windows are 128-aligned so AV reads v8 chunks
    # directly (no shifted copies): lo = 128(t-1) interior.
    hi = min(GW, 8 * t + 13) * GH
    if t == 0:
        lo, nch = 0, 2
    elif t == 7:
        lo, nch = 768, 2
    elif t == 6:
        lo, nch = 640, 3
    else:
        lo, nch = 128 * (t - 1), 4
    return lo, hi, nch


def _build_masks():
    mk = np.zeros((128, NT, 4, 128), dtype=np.float32)
    for t in range(NT):
        lo, hi, nch = _win(t)
        for c in range(nch):
            kl = np.arange(128)
            ql = np.arange(128)
            ka = lo + c * 128 + kl
            qa = t * 128 + ql
            kh, kw = ka % GH, ka // GH
            qh, qw = qa % GH, qa // GH
            ok = (
                (ka[:, None] < hi)
                & (np.abs(kh[:, None] - qh[None, :]) <= HK // 2)
                & (np.abs(kw[:, None] - qw[None, :]) <= WK // 2)
            )
            mk[:, t, c, :] = ok
    return mk.astype(np.float16)


_CACHE = {}


def _build_bass():
    import concourse.tile as tile
    from concourse import bacc, mybir

    f32, f16 = mybir.dt.float32, mybir.dt.float16
    Exp = mybir.ActivationFunctionType.Exp

    nc = bacc.Bacc("TRN2", target_bir_lowering=False)
    xt = nc.dram_tensor("xt", [NB, 512, 1024], f16, kind="ExternalInput")
    wqk = nc.dram_tensor("wqk", [512, 1024], f16, kind="ExternalInput")
    wv = nc.dram_tensor("wv", [512, 512], f16, kind="ExternalInput")
    wp = nc.dram_tensor("wp", [512, 512], f16, kind="ExternalInput")
    bfull = nc.dram_tensor("bfull", [128, 512], f32, kind="ExternalInput")
    maskd = nc.dram_tensor("maskd", [128, NT, 4, 128], f16, kind="ExternalInput")
    seld = nc.dram_tensor("seld", [128, 128], f16, kind="ExternalInput")
    y = nc.dram_tensor("y", [NB, 1024, 512], f32, kind="ExternalOutput")

    with tile.TileContext(nc) as tc:
        with tc.tile_pool(name="const", bufs=1) as const, \
             tc.tile_pool(name="xtp", bufs=2) as xtp, \
             tc.tile_pool(name="qkp", bufs=2) as qkp, \
             tc.tile_pool(name="vp", bufs=2) as vp, \
             tc.tile_pool(name="otp", bufs=2) as otp, \
             tc.tile_pool(name="work", bufs=3) as work, \
             tc.tile_pool(name="yout", bufs=3) as yout, \
             tc.tile_pool(name="psA", bufs=2, space="PSUM") as psA, \
             tc.tile_pool(name="psS", bufs=2, space="PSUM") as psS, \
             tc.tile_pool(name="psV", bufs=2, space="PSUM") as psV:

            # ---- constants ----
            wqk_sb = const.tile([128, 4, 1024], f16)
            nc.gpsimd.dma_start(out=wqk_sb, in_=wqk.rearrange("(k p) m -> p k m", p=128))
            wv_sb = const.tile([128, 4, 512], f16)
            nc.gpsimd.dma_start(out=wv_sb, in_=wv.rearrange("(k p) m -> p k m", p=128))
            wp_sb = const.tile([128, 4, 512], f16)
            nc.gpsimd.dma_start(out=wp_sb, in_=wp.rearrange("(k p) m -> p k m", p=128))
            bias_sb = const.tile([128, 512], f32)
            nc.gpsimd.dma_start(out=bias_sb, in_=bfull[:, :])
            mask_sb = const.tile([128, NT, 4, 128], f16)
            nc.gpsimd.dma_start(out=mask_sb, in_=maskd[:, :, :, :])
            sel_sb = const.tile([128, 128], f16)
            nc.gpsimd.dma_start(out=sel_sb, in_=seld[:, :])
            ones_sb = const.tile([128, 1], f16)
            nc.gpsimd.memset(ones_sb, 1.0)
            ebias_sb = const.tile([128, 1], f32)
            nc.gpsimd.memset(ebias_sb, -2.0)

            for n in range(NB):
                # ---- phase A: projections ----
                xt_sb = xtp.tile([128, 4, 1024], f16, tag="x16")
                nc.gpsimd.dma_start(
                    out=xt_sb, in_=xt[n].rearrange("(k p) s -> p k s", p=128)
                )

                qkT = qkp.tile([128, 8, 1024], f16)
                for m in range(8):
                    for sh in range(2):
                        ps = psA.tile([128, 512], f32, tag="A")
                        for k in range(4):
                            nc.tensor.matmul(
                                ps,
                                wqk_sb[:, k, m * 128:(m + 1) * 128],
                                xt_sb[:, k, sh * 512:(sh + 1) * 512],
                                start=(k == 0),
                                stop=(k == 3),
                            )
                        nc.scalar.copy(
                            out=qkT[:, m, sh * 512:(sh + 1) * 512], in_=ps
                        )

                v8 = vp.tile([128, 8, 512], f16, tag="v8")
                for st in range(8):
                    ps = psA.tile([128, 512], f32, tag="A")
                    for k in range(4):
                        nc.tensor.matmul(
                            ps,
                            xt_sb[:, k, st * 128:(st + 1) * 128],
                            wv_sb[:, k, :],
                            start=(k == 0),
                            stop=(k == 3),
                        )
                    nc.scalar.copy(out=v8[:, st, :], in_=ps)

                # ---- phase B: local attention (k-major scores) ----
                outT = otp.tile([128, 4, 1024], f16)
                for t in range(NT):
                    lo, hi, nch = _win(t)
                    vsrc = v8
                    vb = lo // 128
                    for hp in range(4):
                        ps_sT = psS.tile([128, 2, 4, 128], f32, tag="sT")
                        for ho in range(2):
                            p0 = ho * 64
                            for c in range(nch):
                                nc.tensor.matmul(
                                    ps_sT[:, ho, c, :],
                                    qkT[p0:p0 + 64, 4 + hp, lo + 128 * c:lo + 128 * c + 128],
                                    qkT[p0:p0 + 64, hp, t * 128:(t + 1) * 128],
                                    start=True,
                                    stop=True,
                                )
                        eM16 = work.tile([128, 2, 4, 128], f16, tag="e16")
                        eMm = work.tile([128, 2, 4, 128], f16, tag="eMm")
                        nc.scalar.activation(
                            eM16[:, :, 0:nch, :], ps_sT[:, :, 0:nch, :], Exp,
                            bias=ebias_sb[:, 0:1],
                        )
                        for ho in range(2):
                            nc.gpsimd.tensor_mul(
                                eMm[:, ho, 0:nch, :],
                                eM16[:, ho, 0:nch, :],
                                mask_sb[:, t, 0:nch, :],
                            )
                        # ps_av layout: oT both heads [0:128, 0:128],
                        # z0 [64:65, 128:256], z1 [64:65, 256:384],
                        # rzb [0:128, 384:512]
                        ps_av = psV.tile([128, 512], f32, tag="av")
                        ps_rzb = ps_av[:, 384:512]
                        # NOTE: each PSUM accumulation group must fully
                        # complete before the next group's start=True in the
                        # same bank (start lazily marks the whole 2KB bank
                        # pending-zero on its partitions).
                        for ho in range(2):
                            h = 2 * hp + ho
                            for c in range(nch):
                                nc.tensor.matmul(
                                    ps_av[ho * 64:ho * 64 + 64, 0:128],
                                    vsrc[:, vb + c, h * 64:(h + 1) * 64],
                                    eMm[:, ho, c, :],
                                    start=(c == 0),
                                    stop=(c == nch - 1),
                                    skip_group_check=True,
                                )
                        for ho in range(2):
                            for c in range(nch):
                                nc.tensor.matmul(
                                    ps_av[64:65, 128 + ho * 128:256 + ho * 128],
                                    ones_sb[:, 0:1],
                                    eMm[:, ho, c, :],
                                    start=(c == 0),
                                    stop=(c == nch - 1),
                                    skip_group_check=True,
                                )
                        rz = work.tile([128, 256], f16, tag="rz")
                        with nc.allow_low_precision("softmax 1/z in fp16"):
                            nc.vector.reciprocal(
                                rz[64:65, 0:256], ps_av[64:65, 128:384]
                            )
                        for ho in range(2):
                            nc.tensor.matmul(
                                ps_rzb[ho * 64:ho * 64 + 64, :],
                                sel_sb[64:65, 0:64],
                                rz[64:65, ho * 128:ho * 128 + 128],
                                start=True,
                                stop=True,
                                skip_group_check=True,
                            )
                        rzb_sb = work.tile([128, 128], f16, tag="rzb")
                        nc.scalar.copy(out=rzb_sb, in_=ps_rzb)
                        nc.vector.tensor_mul(
                            outT[:, hp, t * 128:(t + 1) * 128],
                            ps_av[:, 0:128],
                            rzb_sb,
                        )

                # ---- phase C: output projection ----
                for st in range(8):
                    ps_y = psA.tile([128, 512], f32, tag="A")
                    for k in range(4):
                        nc.tensor.matmul(
                            ps_y,
                            outT[:, k, st * 128:(st + 1) * 128],
                            wp_sb[:, k, :],
                            start=(k == 0),
                            stop=(k == 3),
                        )
                    y_sb = yout.tile([128, 512], f32, tag="y")
                    nc.vector.scalar_tensor_tensor(
                        out=y_sb, in0=ps_y, scalar=1.0, in1=bias_sb,
                        op0=mybir.AluOpType.bypass, op1=mybir.AluOpType.add,
                    )
                    nc.gpsimd.dma_start(
                        out=y[n, st * 128:(st + 1) * 128, :], in_=y_sb
                    )

    nc.finalize()
    return nc


def _install_prof_shim():
    import sys
    import types
    if "antenv.axon_hooks" in sys.modules:
        return
    try:
        from trn_agent_boot.trn_boot import _ntff_profile_via_ctypes
        hook = _ntff_profile_via_ctypes("/opt/axon/libaxon_pjrt.so")
    except Exception:
        return
    mod = types.ModuleType("antenv.axon_hooks")
    mod.get_axon_ntff_profile_hook = lambda: hook
    mod.set_axon_ntff_profile_hook = lambda h: None
    sys.modules["antenv.axon_hooks"] = mod


def _prep_inputs(inputs):
    x = np.ascontiguousarray(inputs["x"], dtype=np.float32)
    w_qkv = np.asarray(inputs["W_qkv"], dtype=np.float32)
    w_proj = np.ascontiguousarray(inputs["W_proj"], dtype=np.float32)
    b_proj = np.asarray(inputs["b_proj"], dtype=np.float32)

    wqk = w_qkv[:, : 2 * C].copy()
    wqk[:, :C] *= HD ** -0.5
    wqk = wqk.astype(np.float16)
    wvf = np.ascontiguousarray(w_qkv[:, 2 * C:]).astype(np.float16)
    wpf = w_proj.astype(np.float16)
    bfull = np.tile(b_proj[None, :], (128, 1)).astype(np.float32)
    masks = _build_masks()
    sel = np.zeros((128, 128), dtype=np.float16)
    sel[64, 0:64] = 1.0
    sel[65, 64:128] = 1.0

    # w-major reorder: p = w*16 + h
    xw = x.reshape(N, GH, GW, C).transpose(0, 2, 1, 3).reshape(N, S, C)
    xt16 = np.ascontiguousarray(xw.transpose(0, 2, 1)).astype(np.float16)

    in_maps = []
    for c in range(N_CORES):
        in_maps.append({
            "xt": xt16[c * NB:(c + 1) * NB],
            "wqk": wqk, "wv": wvf, "wp": wpf,
            "bfull": bfull, "maskd": masks, "seld": sel,
        })
    return in_maps


def _unprep_output(y_w):
    # y_w: [N, S(w-major), C] -> h-major
    return (
        y_w.reshape(N, GW, GH, C).transpose(0, 2, 1, 3).reshape(N, S, C)
    )


def _run(inputs, trace=False):
    from concourse.bass_utils import run_bass_kernel_spmd

    if trace:
        _install_prof_shim()
    if "nc" not in _CACHE:
        _CACHE["nc"] = _build_bass()
    nc = _CACHE["nc"]

    in_maps = _prep_inputs(inputs)
    res = run_bass_kernel_spmd(
        nc, in_maps, core_ids=list(range(N_CORES)), trace=trace
    )
    y_w = np.concatenate([res.results[c]["y"] for c in range(N_CORES)], axis=0)
    return _unprep_output(y_w).astype(np.float32), res


def kernel(**inputs):
    out, _ = _run(inputs, trace=False)
    return out


# revision 18
# speedup vs baseline: 1.1789x; 1.1789x over previous
"""Trainium2 Bass kernel for nn_AttnMixer (2D-local sparse attention).

Strategy (v2): data-parallel over batch N=32 across 8 cores (4/core).
W-MAJOR spatial layout: position p = w*16 + h, so the |dw|<=5 band becomes
an 18-column window -> 288-key (384 padded) windows instead of 512, and the
|dh|<=3 band lives entirely inside each 16-high column (pure mask, no
windowing).

Per (q-tile of 128 = 8 w-cols, head-pair): scores are computed TRANSPOSED
(k-major, [kwin, 128q] PSUM chunks) so AV consumes the probabilities
directly -- no PE transposes and no PSUM->SBUF prob copies. exp on ACT
(bias -2), 0/1-mask multiply on Pool, AV matmuls accumulate o^T per head
into a shared PSUM tile, softmax denominators z ride as [1,128] ones-vector
matmuls into spare rows of the same tile, reciprocal on DVE, 1/z broadcast
across partitions via one K=2 matmul, and the normalize is fused into the
single PSUM->SBUF merge per head-pair on DVE. All matmul dtypes fp16
(fp8 tested: exceeds the 2e-2 error budget).
"""

import numpy as np

N_CORES = 8
N, S, C = 32, 1024, 512
NB = N // N_CORES
GH, GW = 16, 64
HK, WK = 7, 11
NH, HD = 8, 64
NT = S // 128  # 8 q-tiles (8 w-cols each)


def _win(t):
    # key window for q-tile t (q w-cols [8t, 8t+8); valid keys w in
    # [8t-5, 8t+13)). Windows are 128-aligned so AV reads v8 chunks
    # directly (no shifted copies): lo = 128(t-1) interior.
    hi = min(GW, 8 * t + 13) * GH
    if t == 0:
        lo, nch = 0, 2
    elif t == 7:
        lo, nch = 768, 2
    elif t == 6:
        lo, nch = 640, 3
    else:
        lo, nch = 128 * (t - 1), 4
    return lo, hi, nch


def _build_masks():
    mk = np.zeros((128, NT, 4, 128), dtype=np.float32)
    for t in range(NT):
        lo, hi, nch = _win(t)
        for c in range(nch):
            kl = np.arange(128)
            ql = np.arange(128)
            ka = lo + c * 128 + kl
            qa = t * 128 + ql
            kh, kw = ka % GH, ka // GH
            qh, qw = qa % GH, qa // GH
            ok = (
                (ka[:, None] < hi)
                & (np.abs(kh[:, None] - qh[None, :]) <= HK // 2)
                & (np.abs(kw[:, None] - qw[None, :]) <= WK // 2)
            )
            mk[:, t, c, :] = ok
    return mk.astype(np.float16)


_CACHE = {}


def _build_bass():
    import concourse.tile as tile
    from concourse import bacc, mybir

    f32, f16 = mybir.dt.float32, mybir.dt.float16
    Exp = mybir.ActivationFunctionType.Exp

    nc = bacc.Bacc("TRN2", target_bir_lowering=False)
    xt = nc.dram_tensor("xt", [NB, 512, 1024], f16, kind="ExternalInput")
    wqk = nc.dram_tensor("wqk", [512, 1024], f16, kind="ExternalInput")
    wv = nc.dram_tensor("wv", [512, 512], f16, kind="ExternalInput")
    wp = nc.dram_tensor("wp", [512, 512], f16, kind="ExternalInput")
    bfull = nc.dram_tensor("bfull", [128, 512], f32, kind="ExternalInput")
    maskd = nc.dram_tensor("maskd", [128, NT, 4, 128], f16, kind="ExternalInput")
    seld = nc.dram_tensor("seld", [128, 128], f16, kind="ExternalInput")
    y = nc.dram_tensor("y", [NB, 1024, 512], f32, kind="ExternalOutput")

    with tile.TileContext(nc) as tc:
        with tc.tile_pool(name="const", bufs=1) as const, \
             tc.tile_pool(name="xtp", bufs=2) as xtp, \
             tc.tile_pool(name="qkp", bufs=2) as qkp, \
             tc.tile_pool(name="vp", bufs=2) as vp, \
             tc.tile_pool(name="otp", bufs=2) as otp, \
             tc.tile_pool(name="work", bufs=3) as work, \
             tc.tile_pool(name="yout", bufs=3) as yout, \
             tc.tile_pool(name="psA", bufs=2, space="PSUM") as psA, \
             tc.tile_pool(name="psS", bufs=2, space="PSUM") as psS, \
             tc.tile_pool(name="psV", bufs=2, space="PSUM") as psV:

            # ---- constants ----
            wqk_sb = const.tile([128, 4, 1024], f16)
            nc.gpsimd.dma_start(out=wqk_sb, in_=wqk.rearrange("(k p) m -> p k m", p=128))
            wv_sb = const.tile([128, 4, 512], f16)
            nc.gpsimd.dma_start(out=wv_sb, in_=wv.rearrange("(k p) m -> p k m", p=128))
            wp_sb = const.tile([128, 4, 512], f16)
            nc.gpsimd.dma_start(out=wp_sb, in_=wp.rearrange("(k p) m -> p k m", p=128))
            bias_sb = const.tile([128, 512], f32)
            nc.gpsimd.dma_start(out=bias_sb, in_=bfull[:, :])
            mask_sb = const.tile([128, NT, 4, 128], f16)
            nc.gpsimd.dma_start(out=mask_sb, in_=maskd[:, :, :, :])
            sel_sb = const.tile([128, 128], f16)
            nc.gpsimd.dma_start(out=sel_sb, in_=seld[:, :])
            ones_sb = const.tile([128, 32], f16)
            nc.gpsimd.memset(ones_sb, 1.0)
            ebias_sb = const.tile([128, 1], f32)
            nc.gpsimd.memset(ebias_sb, -2.0)

            for n in range(NB):
                # ---- phase A: projections ----
                xt_sb = xtp.tile([128, 4, 1024], f16, tag="x16")
                nc.gpsimd.dma_start(
                    out=xt_sb, in_=xt[n].rearrange("(k p) s -> p k s", p=128)
                )

                qkT = qkp.tile([128, 8, 1024], f16)
                for m in range(8):
                    for sh in range(2):
                        ps = psA.tile([128, 512], f32, tag="A")
                        for k in range(4):
                            nc.tensor.matmul(
                                ps,
                                wqk_sb[:, k, m * 128:(m + 1) * 128],
                                xt_sb[:, k, sh * 512:(sh + 1) * 512],
                                start=(k == 0),
                                stop=(k == 3),
                            )
                        nc.scalar.copy(
                            out=qkT[:, m, sh * 512:(sh + 1) * 512], in_=ps
                        )

                v8 = vp.tile([128, 8, 512], f16, tag="v8")
                for st in range(8):
                    ps = psA.tile([128, 512], f32, tag="A")
                    for k in range(4):
                        nc.tensor.matmul(
                            ps,
                            xt_sb[:, k, st * 128:(st + 1) * 128],
                            wv_sb[:, k, :],
                            start=(k == 0),
                            stop=(k == 3),
                        )
                    nc.scalar.copy(out=v8[:, st, :], in_=ps)

                # ---- phase B: local attention (k-major scores) ----
                outT = otp.tile([128, 4, 1024], f16)
                for t in range(NT):
                    lo, hi, nch = _win(t)
                    vsrc = v8
                    vb = lo // 128
                    for hp in range(4):
                        ps_sT = psS.tile([128, 2, 4, 128], f32, tag="sT")
                        for ho in range(2):
                            p0 = ho * 64
                            for c in range(nch):
                                nc.tensor.matmul(
                                    ps_sT[:, ho, c, :],
                                    qkT[p0:p0 + 64, 4 + hp, lo + 128 * c:lo + 128 * c + 128],
                                    qkT[p0:p0 + 64, hp, t * 128:(t + 1) * 128],
                                    start=True,
                                    stop=True,
                                )
                        eM16 = work.tile([128, 2, 4, 128], f16, tag="e16")
                        eMm = work.tile([128, 2, 4, 128], f16, tag="eMm")
                        nc.scalar.activation(
                            eM16[:, :, 0:nch, :], ps_sT[:, :, 0:nch, :], Exp,
                            bias=ebias_sb[:, 0:1],
                        )
                        nc.vector.tensor_mul(
                            eMm[:, 0, 0:nch, :],
                            eM16[:, 0, 0:nch, :],
                            mask_sb[:, t, 0:nch, :],
                        )
                        nc.gpsimd.tensor_mul(
                            eMm[:, 1, 0:nch, :],
                            eM16[:, 1, 0:nch, :],
                            mask_sb[:, t, 0:nch, :],
                        )
                        # ps_av layout: oT both heads [0:128, 0:128],
                        # z0 [64:65, 128:256], z1 [64:65, 256:384],
                        # rzb [0:128, 384:512]
                        ps_av = psV.tile([128, 512], f32, tag="av")
                        ps_rzb = ps_av[:, 384:512]
                        # NOTE: each PSUM accumulation group must fully
                        # complete before the next group's start=True in the
                        # same bank (start lazily marks the whole 2KB bank
                        # pending-zero on its partitions).
                        for ho in range(2):
                            h = 2 * hp + ho
                            for c in range(nch):
                                nc.tensor.matmul(
                                    ps_av[ho * 64:ho * 64 + 64, 0:128],
                                    vsrc[:, vb + c, h * 64:(h + 1) * 64],
                                    eMm[:, ho, c, :],
                                    start=(c == 0),
                                    stop=(c == nch - 1),
                                    skip_group_check=True,
                                )
                        # z replicated on 32 partition rows (M=32, same PE
                        # cost as M=1) so the reciprocal runs on 32 lanes.
                        for c in range(nch):
                            nc.tensor.matmul(
                                ps_av[64:96, 128:384],
                                ones_sb[:, 0:32],
                                eMm[:, :, c, :],
                                start=(c == 0),
                                stop=(c == nch - 1),
                                skip_group_check=True,
                            )
                        rz = work.tile([128, 256], f16, tag="rz")
                        with nc.allow_low_precision("softmax 1/z in fp16"):
                            nc.vector.reciprocal(
                                rz[64:96, 0:256], ps_av[64:96, 128:384]
                            )
                        for ho in range(2):
                            nc.tensor.matmul(
                                ps_rzb[ho * 64:ho * 64 + 64, :],
                                sel_sb[64:65, 0:64],
                                rz[64:65, ho * 128:ho * 128 + 128],
                                start=True,
                                stop=True,
                                skip_group_check=True,
                            )
                        rzb_sb = work.tile([128, 128], f16, tag="rzb")
                        nc.scalar.copy(out=rzb_sb, in_=ps_rzb)
                        nc.vector.tensor_mul(
                            outT[:, hp, t * 128:(t + 1) * 128],
                            ps_av[:, 0:128],
                            rzb_sb,
                        )

                # ---- phase C: output projection ----
                for st in range(8):
                    ps_y = psA.tile([128, 512], f32, tag="A")
                    for k in range(4):
                        nc.tensor.matmul(
                            ps_y,
                            outT[:, k, st * 128:(st + 1) * 128],
                            wp_sb[:, k, :],
                            start=(k == 0),
                            stop=(k == 3),
                        )
                    y_sb = yout.tile([128, 512], f32, tag="y")
                    nc.vector.scalar_tensor_tensor(
                        out=y_sb, in0=ps_y, scalar=1.0, in1=bias_sb,
                        op0=mybir.AluOpType.bypass, op1=mybir.AluOpType.add,
                    )
                    nc.gpsimd.dma_start(
                        out=y[n, st * 128:(st + 1) * 128, :], in_=y_sb
                    )

    nc.finalize()
    return nc


def _install_prof_shim():
    import sys
    import types
    if "antenv.axon_hooks" in sys.modules:
        return
    try:
        from trn_agent_boot.trn_boot import _ntff_profile_via_ctypes
        hook = _ntff_profile_via_ctypes("/opt/axon/libaxon_pjrt.so")
    except Exception:
        return
    mod = types.ModuleType("antenv.axon_hooks")
    mod.get_axon_ntff_profile_hook = lambda: hook
    mod.set_axon_ntff_profile_hook = lambda h: None
    sys.modules["antenv.axon_hooks"] = mod


def _prep_inputs(inputs):
    x = np.ascontiguousarray(inputs["x"], dtype=np.float32)
    w_qkv = np.asarray(inputs["W_qkv"], dtype=np.float32)
    w_proj = np.ascontiguousarray(inputs["W_proj"], dtype=np.float32)
    b_proj = np.asarray(inputs["b_proj"], dtype=np.float32)

    wqk = w_qkv[:, : 2 * C].copy()
    wqk[:, :C] *= HD ** -0.5
    wqk = wqk.astype(np.float16)
    wvf = np.ascontiguousarray(w_qkv[:, 2 * C:]).astype(np.float16)
    wpf = w_proj.astype(np.float16)
    bfull = np.tile(b_proj[None, :], (128, 1)).astype(np.float32)
    masks = _build_masks()
    sel = np.zeros((128, 128), dtype=np.float16)
    sel[64, 0:64] = 1.0
    sel[65, 64:128] = 1.0

    # w-major reorder: p = w*16 + h
    xw = x.reshape(N, GH, GW, C).transpose(0, 2, 1, 3).reshape(N, S, C)
    xt16 = np.ascontiguousarray(xw.transpose(0, 2, 1)).astype(np.float16)

    in_maps = []
    for c in range(N_CORES):
        in_maps.append({
            "xt": xt16[c * NB:(c + 1) * NB],
            "wqk": wqk, "wv": wvf, "wp": wpf,
            "bfull": bfull, "maskd": masks, "seld": sel,
        })
    return in_maps


def _unprep_output(y_w):
    # y_w: [N, S(w-major), C] -> h-major
    return (
        y_w.reshape(N, GW, GH, C).transpose(0, 2, 1, 3).reshape(N, S, C)
    )


def _run(inputs, trace=False):
    from concourse.bass_utils import run_bass_kernel_spmd

    if trace:
        _install_prof_shim()
    if "nc" not in _CACHE:
        _CACHE["nc"] = _build_bass()
    nc = _CACHE["nc"]

    in_maps = _prep_inputs(inputs)
    res = run_bass_kernel_spmd(
        nc, in_maps, core_ids=list(range(N_CORES)), trace=trace
    )
    y_w = np.concatenate([res.results[c]["y"] for c in range(N_CORES)], axis=0)
    return _unprep_output(y_w).astype(np.float32), res


def kernel(**inputs):
    out, _ = _run(inputs, trace=False)
    return out


# revision 25
# speedup vs baseline: 1.4708x; 1.2476x over previous
"""Trainium2 Bass kernel for nn_AttnMixer (2D-local sparse attention).

Strategy: data-parallel over batch N=32 across 8 cores (4 batches/core).
Per core, per batch:
  A) qkT[2C, S] = Wqk.T @ xT and v[S, C], fp16 operands / fp32 PSUM
     (q pre-scaled by hd^-0.5 host-side).
  B) per (q-tile of 128, head-pair): scores = qT.T @ kT_window (K=64
     row-packed pairs into one [128,1024] PSUM), exp on ACT -> fp16,
     binary-mask multiply + fused row-sum on DVE, normalize, PE-transpose
     probs, AV matmuls col-packed per head pair accumulating
     o^T[d, qi] -> out^T[C, S].
  C) proj: y = outT.T @ Wp, bias fused into the PSUM->SBUF copy, DMA out.
All shapes hardcoded; host side only reshapes/transposes numpy inputs.
"""

import numpy as np

N_CORES = 8
N, S, C = 32, 1024, 512
NB = N // N_CORES
GH, GW = 16, 64
HK, WK = 7, 11
NH, HD = 8, 64
NT = S // 128  # 8 q-tiles per (n, h)


def _win_start(t):
    return min(max(128 * t - 192, 0), 512)


def _build_masks():
    mk = np.zeros((NT, 128, 512), dtype=np.float32)
    for t in range(NT):
        stk = _win_start(t)
        qs = t * 128 + np.arange(128)
        ks = stk + np.arange(512)
        qh, qw = qs // GW, qs % GW
        kh, kw = ks // GW, ks % GW
        ok = (np.abs(qh[:, None] - kh[None, :]) <= HK // 2) & (
            np.abs(qw[:, None] - kw[None, :]) <= WK // 2
        )
        mk[t] = ok
    return mk.astype(np.float16)


_CACHE = {}


def _build_bass():
    import concourse.tile as tile
    from concourse import bacc, mybir

    f32, f16 = mybir.dt.float32, mybir.dt.float16

    nc = bacc.Bacc("TRN2", target_bir_lowering=False)
    xt = nc.dram_tensor("xt", [NB, 512, 1024], f16, kind="ExternalInput")
    wqk = nc.dram_tensor("wqk", [512, 1024], f16, kind="ExternalInput")
    wv = nc.dram_tensor("wv", [512, 512], f16, kind="ExternalInput")
    wp = nc.dram_tensor("wp", [512, 512], f16, kind="ExternalInput")
    bfull = nc.dram_tensor("bfull", [128, 512], f32, kind="ExternalInput")
    identd = nc.dram_tensor("identd", [128, 128], f16, kind="ExternalInput")
    maskd = nc.dram_tensor("maskd", [NT, 128, 512], f16, kind="ExternalInput")
    y = nc.dram_tensor("y", [NB, 1024, 512], f32, kind="ExternalOutput")

    with tile.TileContext(nc) as tc:
        with tc.tile_pool(name="const", bufs=1) as const, \
             tc.tile_pool(name="xtp", bufs=2) as xtp, \
             tc.tile_pool(name="qkp", bufs=2) as qkp, \
             tc.tile_pool(name="vp", bufs=3) as vp, \
             tc.tile_pool(name="otp", bufs=3) as otp, \
             tc.tile_pool(name="work", bufs=6) as work, \
             tc.tile_pool(name="yout", bufs=3) as yout, \
             tc.tile_pool(name="psA", bufs=2, space="PSUM") as psA, \
             tc.tile_pool(name="psT", bufs=2, space="PSUM") as psT:

            # ---- constants ----
            wqk_sb = const.tile([128, 4, 1024], f16)
            nc.gpsimd.dma_start(out=wqk_sb, in_=wqk.rearrange("(k p) m -> p k m", p=128))
            wv_sb = const.tile([128, 4, 512], f16)
            nc.gpsimd.dma_start(out=wv_sb, in_=wv.rearrange("(k p) m -> p k m", p=128))
            wp_sb = const.tile([128, 4, 512], f16)
            nc.gpsimd.dma_start(out=wp_sb, in_=wp.rearrange("(k p) m -> p k m", p=128))
            bias_sb = const.tile([128, 512], f32)
            nc.gpsimd.dma_start(out=bias_sb, in_=bfull[:, :])
            ident_sb = const.tile([128, 128], f16)
            nc.gpsimd.dma_start(out=ident_sb, in_=identd[:, :])
            mask_sb = const.tile([128, NT, 512], f16)
            nc.gpsimd.dma_start(out=mask_sb, in_=maskd.rearrange("t p m -> p t m"))

            for n in range(NB):
                # ---- phase A: projections ----
                xt_sb = xtp.tile([128, 4, 1024], f16)
                nc.gpsimd.dma_start(
                    out=xt_sb, in_=xt[n].rearrange("(k p) s -> p k s", p=128)
                )

                qkT = qkp.tile([128, 8, 1024], f16)
                for m in range(8):
                    ps = psA.tile([128, 1024], f32, tag="A")
                    for k in range(4):
                        for sh in range(2):
                            nc.tensor.matmul(
                                ps[:, sh * 512:(sh + 1) * 512],
                                wqk_sb[:, k, m * 128:(m + 1) * 128],
                                xt_sb[:, k, sh * 512:(sh + 1) * 512],
                                start=(k == 0),
                                stop=(k == 3),
                            )
                    nc.any.tensor_copy(qkT[:, m, :], ps)

                v_ev = vp.tile([128, 8, 512], f16, tag="ve")
                v_od = vp.tile([128, 7, 512], f16, tag="vo")
                for st in range(8):
                    ps = psA.tile([128, 512], f32, tag="A")
                    for k in range(4):
                        nc.tensor.matmul(
                            ps,
                            xt_sb[:, k, st * 128:(st + 1) * 128],
                            wv_sb[:, k, :],
                            start=(k == 0),
                            stop=(k == 3),
                        )
                    nc.any.tensor_copy(v_ev[:, st, :], ps)
                for m in range(7):
                    nc.gpsimd.dma_start(out=v_od[0:64, m, :], in_=v_ev[64:128, m, :])
                    nc.gpsimd.dma_start(out=v_od[64:128, m, :], in_=v_ev[0:64, m + 1, :])

                # ---- phase B: local attention ----
                outT = otp.tile([128, 4, 1024], f16)
                for t in range(NT):
                    stk = _win_start(t)
                    if stk % 128 == 0:
                        vsrc, vbase = v_ev, stk // 128
                    else:
                        vsrc, vbase = v_od, (stk - 64) // 128
                    for hp in range(4):
                        # paired scores: head 2hp on array rows 0:64 ->
                        # bank cols 0:512, head 2hp+1 on rows 64:128 ->
                        # cols 512:1024 (different PSUM banks, concurrent)
                        ps_s = psA.tile([128, 1024], f32, tag="A")
                        for ho in range(2):
                            p0 = ho * 64
                            nc.tensor.matmul(
                                ps_s[:, ho * 512:(ho + 1) * 512],
                                qkT[p0:p0 + 64, hp, t * 128:(t + 1) * 128],
                                qkT[p0:p0 + 64, 4 + hp, stk:stk + 512],
                                start=True,
                                stop=True,
                            )
                        e = work.tile([128, 1024], f16, tag="e")
                        nc.scalar.activation(e, ps_s, mybir.ActivationFunctionType.Exp)

                        ps_o = psT.tile([128, 128], f32, tag="oT")
                        for ho in range(2):
                            h = 2 * hp + ho
                            eM = work.tile([128, 512], f16, tag=f"eM{ho}")
                            z = work.tile([128, 1], f32, tag=f"z{ho}")
                            nc.vector.scalar_tensor_tensor(
                                out=eM, in0=e[:, ho * 512:(ho + 1) * 512],
                                scalar=1.0, in1=mask_sb[:, t, :],
                                op0=mybir.AluOpType.bypass,
                                op1=mybir.AluOpType.mult,
                                accum_out=z,
                            )
                            rz = work.tile([128, 1], f32, tag=f"rz{ho}")
                            nc.vector.reciprocal(rz, z)
                            probs = work.tile([128, 512], f16, tag=f"probs{ho}")
                            nc.vector.tensor_scalar_mul(probs, in0=eM, scalar1=rz)
                            ps_t = psT.tile([128, 512], f16, tag="aT")
                            for j in range(4):
                                nc.tensor.transpose(
                                    ps_t[:, j * 128:(j + 1) * 128],
                                    probs[:, j * 128:(j + 1) * 128],
                                    ident_sb,
                                )
                            aT = work.tile([128, 512], f16, tag=f"aTs{ho}")
                            nc.any.tensor_copy(aT, ps_t)
                            for j in range(4):
                                nc.tensor.matmul(
                                    ps_o[ho * 64:(ho + 1) * 64, :],
                                    vsrc[:, vbase + j, h * 64:(h + 1) * 64],
                                    aT[:, j * 128:(j + 1) * 128],
                                    start=(j == 0),
                                    stop=(j == 3),
                                    skip_group_check=True,
                                )
                        nc.any.tensor_copy(outT[:, hp, t * 128:(t + 1) * 128], ps_o)

                # ---- phase C: output projection ----
                for st in range(8):
                    ps_y = psA.tile([128, 512], f32, tag="A")
                    for k in range(4):
                        nc.tensor.matmul(
                            ps_y,
                            outT[:, k, st * 128:(st + 1) * 128],
                            wp_sb[:, k, :],
                            start=(k == 0),
                            stop=(k == 3),
                        )
                    y_sb = yout.tile([128, 512], f32, tag="y")
                    nc.vector.scalar_tensor_tensor(
                        out=y_sb, in0=ps_y, scalar=1.0, in1=bias_sb,
                        op0=mybir.AluOpType.bypass, op1=mybir.AluOpType.add,
                    )
                    nc.gpsimd.dma_start(
                        out=y[n, st * 128:(st + 1) * 128, :], in_=y_sb
                    )

    nc.finalize()
    return nc


def _install_prof_shim():
    import sys
    import types
    if "antenv.axon_hooks" in sys.modules:
        return
    try:
        from trn_agent_boot.trn_boot import _ntff_profile_via_ctypes
        hook = _ntff_profile_via_ctypes("/opt/axon/libaxon_pjrt.so")
    except Exception:
        return
    mod = types.ModuleType("antenv.axon_hooks")
    mod.get_axon_ntff_profile_hook = lambda: hook
    mod.set_axon_ntff_profile_hook = lambda h: None
    sys.modules["antenv.axon_hooks"] = mod


def _run(inputs, trace=False):
    from concourse.bass_utils import run_bass_kernel_spmd

    if trace:
        _install_prof_shim()
    if "nc" not in _CACHE:
        _CACHE["nc"] = _build_bass()
    nc = _CACHE["nc"]

    x = np.ascontiguousarray(inputs["x"], dtype=np.float32)
    w_qkv = np.asarray(inputs["W_qkv"], dtype=np.float32)
    w_proj = np.ascontiguousarray(inputs["W_proj"], dtype=np.float32)
    b_proj = np.asarray(inputs["b_proj"], dtype=np.float32)

    wqk = w_qkv[:, : 2 * C].copy()
    wqk[:, :C] *= HD ** -0.5
    wqk = wqk.astype(np.float16)
    wv = np.ascontiguousarray(w_qkv[:, 2 * C:]).astype(np.float16)
    wpf = w_proj.astype(np.float16)
    bfull = np.tile(b_proj[None, :], (128, 1)).astype(np.float32)
    ident = np.eye(128, dtype=np.float16)
    masks = _build_masks()

    xtn = x.transpose(0, 2, 1).astype(np.float16)  # [N, C, S]
    in_maps = []
    for c in range(N_CORES):
        in_maps.append({
            "xt": np.ascontiguousarray(xtn[c * NB:(c + 1) * NB]),
            "wqk": wqk, "wv": wv, "wp": wpf,
            "bfull": bfull, "identd": ident, "maskd": masks,
        })

    res = run_bass_kernel_spmd(
        nc, in_maps, core_ids=list(range(N_CORES)), trace=trace
    )
    out = np.concatenate([res.results[c]["y"] for c in range(N_CORES)], axis=0)
    return out.astype(np.float32), res


def kernel(**inputs):
    out, _ = _run(inputs, trace=False)
    return out

